# revision 1
# baseline (speedup 1.0000x reference)
"""MentionScore fused Bass kernel for 8 Trainium2 NeuronCores.

Strategy (self-contained, hardcoded for the nn_MentionScore problem):
  - Spans are bucketed by start//6250 -> one bucket per core; each core only
    needs its 6250-token slice (+9 halo), so states/embeds are sharded with
    no collectives.
  - Token phase (feature-major): per-token attention-logit MLP -> e = exp(a);
    the span MLP's first layer is decomposed through the gathers:
        g @ sw1 = A[start] + B[end] + (pooled @ sw1_p) + width-term
    with A = states@sw1[0:400], B = states@sw1[400:800], and
    pooled @ sw1_p = (P[end] - P[start-1]) / (Pe[end] - Pe[start-1]) where
    [P | Pe] = inclusive prefix sums of [e*EC | e], EC = embeds@sw1[800:1150].
    Prefix sums run on the vector engine (tensor_tensor_scan along tokens).
  - Tables are written token-major (via PE transposes):
        tab1[t] = [A[t] | [P|Pe][t-1]]   tabB[t] = B[t]
    so one two-index gather of tab1 at (start, end+1) yields A[start], the
    exclusive prefix at start and the inclusive prefix through end.
  - Span phase: 2 indirect-DMA gathers per 128-span tile + small elementwise
    + the [150x150] second layer on PE; the width-bin embedding enters via a
    5-column multi-hot matmul against a host-folded difference table
    (widths 1..10 only reach bins 1,2,3,4,8).
"""

import ml_dtypes
import numpy as np

BF16NP = ml_dtypes.bfloat16

# ---- problem constants (hardcoded per contract) ----
T, S = 50000, 100000
DS, DE, H, DW = 400, 350, 150, 20
W_MAX = 10
BINS5 = np.array([1, 2, 3, 4, 8], np.int64)
NCORES = 8
TPC = T // NCORES            # 6250 tokens per core bucket
TL_PAD = 6272                # 49 * 128 padded local tokens (6250 + 9 halo -> 6259)
TBLR = 6400                  # table rows (>= TL_PAD + 1 for the shifted write)
NT_S = 102                   # span tiles per core
SMAX = NT_S * 128            # 13056 padded spans per core (observed max 12737)
TOK_BLOCKS = [(i * 512, 512) for i in range(12)] + [(6144, 128)]
SPLIT = 3584                 # token boundary: blocks 0..6 cover [0, 3584)
NTA = 44                     # span tiles gathering only rows < SPLIT (early)
TBLE = 3712                  # early-table rows
K400 = [(0, 128), (128, 256), (256, 384), (384, 400)]
K350 = [(0, 128), (128, 256), (256, 350)]
K150 = [(0, 128), (128, 150)]

_PROGRAM_CACHE = {}


def _build_program():
    import concourse.bacc as bacc
    import concourse.bass as bass
    import concourse.mybir as mybir
    import concourse.tile as tile
    from concourse.masks import make_identity

    F32 = mybir.dt.float32
    BF16 = mybir.dt.bfloat16
    I32 = mybir.dt.int32
    AF = mybir.ActivationFunctionType
    OP = mybir.AluOpType

    nc = bacc.Bacc("TRN2", num_devices=NCORES)

    # ---- I/O ----
    # packed [states.T (400, pad to 512) | embeds.T (350, pad to 384)] = 896 rows
    seTd = nc.dram_tensor("seT", [896, TL_PAD], BF16, kind="ExternalInput")
    wk4d = [nc.dram_tensor(f"wk4_{i}", [k1 - k0, 480], BF16, kind="ExternalInput")
            for i, (k0, k1) in enumerate(K400)]
    wk3d = [nc.dram_tensor(f"wk3_{i}", [k1 - k0, 150], BF16, kind="ExternalInput")
            for i, (k0, k1) in enumerate(K350)]
    wk1d = [nc.dram_tensor(f"wk1_{i}", [k1 - k0, 302], BF16, kind="ExternalInput")
            for i, (k0, k1) in enumerate(K150)]
    dtabd = nc.dram_tensor("dtab", [5, H], BF16, kind="ExternalInput")
    b1d = nc.dram_tensor("bias1", [128, 8], F32, kind="ExternalInput")
    scombd = nc.dram_tensor("scomb", [128, 2 * NT_S], I32, kind="ExternalInput")
    endsd = nc.dram_tensor("ends", [128, NT_S], I32, kind="ExternalInput")
    mhd = nc.dram_tensor("mh", [5, SMAX], BF16, kind="ExternalInput")
    scoresd = nc.dram_tensor("scores", [1, SMAX], F32, kind="ExternalOutput")

    # tab1 row t = [A[t] (bf16 x150) | C[t-1] (f32 x151 stored as bf16 x302)]
    # (C shifted): gathered at `start` it gives A[start] + the exclusive
    # prefix; at `end+1` the inclusive prefix. The f32 prefix block is
    # accessed through bitcast views.
    # split into early (rows < SPLIT, complete after token block 6) and late
    # tensors so early span tiles can overlap the remaining token blocks.
    tabE = nc.dram_tensor("tabE", [TBLE, 452], BF16)
    tabL = nc.dram_tensor("tabL", [TBLR, 452], BF16)
    tabBE = nc.dram_tensor("tabBE", [TBLE, 150], BF16)
    tabBL = nc.dram_tensor("tabBL", [TBLR, 150], BF16)

    with tile.TileContext(nc) as tc:
        with (
            tc.tile_pool(name="wpool", bufs=1) as wp,
            tc.tile_pool(name="tok", bufs=3) as tok,
            tc.tile_pool(name="span", bufs=4) as sp,
            tc.tile_pool(name="ps", bufs=7, space="PSUM") as ps,
            tc.tile_pool(name="psc", bufs=1, space="PSUM") as psc,
        ):
            # ---- resident weights / constants ----
            def wload(src, shape, name, dt=F32):
                t = wp.tile(shape, dt, name=name)
                nc.sync.dma_start(t[:], src)
                return t

            wk4 = [wload(wk4d[i][:, :], [k1 - k0, 480], f"wk4s_{i}", BF16) for i, (k0, k1) in enumerate(K400)]
            wk3 = [wload(wk3d[i][:, :], [k1 - k0, 150], f"wk3s_{i}", BF16) for i, (k0, k1) in enumerate(K350)]
            wk1 = [wload(wk1d[i][:, :], [k1 - k0, 302], f"wk1s_{i}", BF16) for i, (k0, k1) in enumerate(K150)]
            w_dt = wload(dtabd[:, :], [5, H], "wdt", BF16)
            b1 = wload(b1d[:, :], [128, 8], "b1")

            w_aw1 = [w[:, 0:128] for w in wk4]
            w_sa = [w[:, 128:256] for w in wk4]
            w_sb = [w[:, 256:384] for w in wk4]
            w_l4 = [w[:, 384:480] for w in wk4]
            w_pm = [w[:, 0:128] for w in wk3]
            w_pl = [w[:, 128:150] for w in wk3]
            w_a2m = [w[:, 0:128] for w in wk1]
            w_a2l = [w[:, 128:150] for w in wk1]
            w_s2m = [w[:, 150:278] for w in wk1]
            w_s2l = [w[:, 278:300] for w in wk1]
            w_a3 = [w[:, 300:301] for w in wk1]
            w_s3 = [w[:, 301:302] for w in wk1]

            scomb_sb = wp.tile([128, 2 * NT_S], I32, name="scomb_sb")
            nc.sync.dma_start(scomb_sb[:], scombd[:, :])
            ends_sb = wp.tile([128, NT_S], I32, name="ends_sb")
            nc.sync.dma_start(ends_sb[:], endsd[:, :])
            mhs_sb = wp.tile([5, SMAX], BF16, name="mhs_sb")
            nc.sync.dma_start(mhs_sb[:], mhd[:, :])

            ones1 = wp.tile([1, 128], F32, name="ones1")
            nc.vector.memset(ones1[:], 1.0)
            ident = wp.tile([128, 128], F32, name="ident")
            make_identity(nc, ident[:])
            identb = wp.tile([128, 128], BF16, name="identb")
            make_identity(nc, identb[:])
            zrow = wp.tile([1, 302], BF16, name="zrow")
            nc.vector.memset(zrow[:], 0.0)
            nc.scalar.dma_start(tabE[0:1, 150:452], zrow[:])
            nc.sync.dma_start(tabL[0:1, 150:452], zrow[:])

            # ================= span phase =================
            scstate = {"pSC": None, "scs": None}

            def span_tile(k, tb1, tbB):
                g, j = divmod(k, 4)

                # two single-index gathers into plain offset-0 tiles: both the
                # 2-index form and sliced-output forms mis-execute in the HW
                # SWDGE ucode (CoreSim models them fine)
                G1a = sp.tile([128, 452], BF16, name="G1a", tag="G1a")
                nc.gpsimd.indirect_dma_start(
                    out=G1a[:], out_offset=None, in_=tb1[:, :],
                    in_offset=bass.IndirectOffsetOnAxis(
                        ap=scomb_sb[:, 2 * k:2 * k + 1], axis=0))
                G1b = sp.tile([128, 452], BF16, name="G1b", tag="G1b")
                nc.gpsimd.indirect_dma_start(
                    out=G1b[:], out_offset=None, in_=tb1[:, :],
                    in_offset=bass.IndirectOffsetOnAxis(
                        ap=scomb_sb[:, 2 * k + 1:2 * k + 2], axis=0))
                GB = sp.tile([128, 150], BF16, name="GB", tag="GB")
                nc.gpsimd.indirect_dma_start(
                    out=GB[:], out_offset=None, in_=tbB[:, :],
                    in_offset=bass.IndirectOffsetOnAxis(
                        ap=ends_sb[:, k:k + 1], axis=0))

                # psw accumulates width-term + A[start] + B[end] on PE
                # (identity-matmul adds the gathered bf16 rows into PSUM)
                psw = ps.tile([128, H], F32, name="psw", tag="ps")
                nc.tensor.matmul(psw[:], lhsT=mhs_sb[:, k * 128:(k + 1) * 128],
                                 rhs=w_dt[:], start=True, stop=False)
                nc.tensor.matmul(psw[:], lhsT=identb[:], rhs=G1a[:, 0:150],
                                 start=False, stop=False)
                nc.tensor.matmul(psw[:], lhsT=identb[:], rhs=GB[:],
                                 start=False, stop=True)

                c0v = G1a[:, 150:452].bitcast(F32)
                c1v = G1b[:, 150:452].bitcast(F32)
                diff = sp.tile([128, 151], F32, name="diff", tag="diff")
                nc.vector.tensor_sub(diff[:], c1v, c0v)
                rec = sp.tile([128, 1], F32, name="rec", tag="rec")
                nc.vector.reciprocal(rec[:], diff[:, 150:151])
                t1 = sp.tile([128, H], F32, name="t1", tag="t1")
                nc.vector.tensor_scalar_mul(t1[:], diff[:, 0:150], rec[:, 0:1])
                h1p = sp.tile([128, H], F32, name="h1p", tag="h1p")
                nc.vector.tensor_add(h1p[:], t1[:], psw[:])
                h1s = sp.tile([128, H], BF16, name="h1s", tag="h1s")
                nc.vector.tensor_scalar(h1s[:], h1p[:], 0.0, None, op0=OP.max)

                pT = ps.tile([128, 256], BF16, name="pT", tag="ps")
                nc.tensor.transpose(pT[:, 0:128], h1s[:, 0:128], identb[:])
                nc.tensor.transpose(pT[0:22, 128:256], h1s[:, 128:150], identb[:])
                h1t1 = sp.tile([128, 128], BF16, name="h1t1", tag="h1t1")
                nc.any.tensor_copy(h1t1[:], pT[:, 0:128])
                h1t2 = sp.tile([22, 128], BF16, name="h1t2", tag="h1t2")
                nc.any.tensor_copy(h1t2[:], pT[0:22, 128:256])

                pH2 = ps.tile([128, 256], F32, name="pH2", tag="ps")
                nc.tensor.matmul(pH2[:, 0:128], lhsT=w_s2m[0], rhs=h1t1[:], start=True, stop=False)
                nc.tensor.matmul(pH2[:, 0:128], lhsT=w_s2m[1], rhs=h1t2[:], start=False, stop=True)
                nc.tensor.matmul(pH2[0:22, 128:256], lhsT=w_s2l[0], rhs=h1t1[:], start=True, stop=False)
                nc.tensor.matmul(pH2[0:22, 128:256], lhsT=w_s2l[1], rhs=h1t2[:], start=False, stop=True)

                h2t1 = sp.tile([128, 128], BF16, name="h2t1", tag="h2t1")
                nc.scalar.activation(h2t1[:], pH2[:, 0:128], AF.Relu, bias=b1[0:128, 4:5])
                h2t2 = sp.tile([22, 128], BF16, name="h2t2", tag="h2t2")
                nc.scalar.activation(h2t2[:], pH2[0:22, 128:256], AF.Relu, bias=b1[0:22, 5:6])

                if j == 0:
                    scstate["pSC"] = psc.tile([1, 512], F32, name="pSC", tag="sc")
                pSC = scstate["pSC"]
                sl = pSC[0:1, j * 128:(j + 1) * 128]
                nc.tensor.matmul(sl, lhsT=w_s3[0], rhs=h2t1[:], start=True, stop=False)
                nc.tensor.matmul(sl, lhsT=w_s3[1], rhs=h2t2[:], start=False, stop=True)

                if j == 3 or k == NT_S - 1:
                    width = (j + 1) * 128
                    if g % 4 == 0:
                        scstate["scs"] = sp.tile([1, 2048], F32, name="scs", tag="scs")
                    scs = scstate["scs"]
                    off = (g % 4) * 512
                    nc.vector.tensor_scalar(scs[0:1, off:off + width],
                                            pSC[0:1, 0:width],
                                            b1[0:1, 6:7], None, op0=OP.add)
                    if g % 4 == 3 or k == NT_S - 1:
                        lo = (g // 4) * 2048
                        w = off + width
                        nc.sync.dma_start(scoresd[0:1, lo:lo + w], scs[0:1, 0:w])

            def emit_span_tiles(ks, tb1, tbB):
                for k in ks:
                    span_tile(k, tb1, tbB)

            # ================= token phase =================
            prevC1 = prevC2 = None
            prevTB = 0
            for bi, (t0, TB) in enumerate(TOK_BLOCKS):
                w1, wB = (tabE, tabBE) if bi <= 6 else (tabL, tabBL)
                seb = tok.tile([128, 7, TB], BF16, name="seb", tag="seb")
                nc.sync.dma_start(
                    seb[:], seTd[:, t0:t0 + TB].rearrange("(j p) t -> p j t", p=128))
                st = [seb[0:128, 0, :], seb[0:128, 1, :], seb[0:128, 2, :],
                      seb[0:16, 3, :]]
                et = [seb[0:128, 4, :], seb[0:128, 5, :], seb[0:94, 6, :]]

                def mm_group(shape, lhs_list, rhs_list, name):
                    p = ps.tile(shape, F32, name=name, tag="ps")
                    n = len(lhs_list)
                    for i in range(n):
                        nc.tensor.matmul(p[:], lhsT=lhs_list[i], rhs=rhs_list[i],
                                         start=(i == 0), stop=(i == n - 1))
                    return p

                pH1 = mm_group([128, TB], w_aw1, st, "pH1")
                pA = mm_group([128, TB], w_sa, st, "pA")
                pB = mm_group([128, TB], w_sb, st, "pB")
                pL = mm_group([96, TB], w_l4, st, "pL")
                pEC = mm_group([128, TB], w_pm, et, "pEC")
                pECl = mm_group([22, TB], w_pl, et, "pECl")

                h1a = tok.tile([128, TB], BF16, name="h1a", tag="h1a")
                nc.scalar.activation(h1a[:], pH1[:], AF.Relu, bias=b1[0:128, 0:1])
                h1b = tok.tile([22, TB], BF16, name="h1b", tag="h1b")
                nc.scalar.activation(h1b[:], pL[0:22, :], AF.Relu, bias=b1[0:22, 1:2])

                pH2 = mm_group([128, TB], w_a2m, [h1a[:], h1b[:]], "pH2")
                pH2l = mm_group([22, TB], w_a2l, [h1a[:], h1b[:]], "pH2l")
                h2a = tok.tile([128, TB], BF16, name="h2a", tag="h2a")
                nc.scalar.activation(h2a[:], pH2[:], AF.Relu, bias=b1[0:128, 2:3])
                h2b = tok.tile([22, TB], BF16, name="h2b", tag="h2b")
                nc.scalar.activation(h2b[:], pH2l[:], AF.Relu, bias=b1[0:22, 3:4])

                pAt = mm_group([1, TB], w_a3, [h2a[:], h2b[:]], "pAt")
                e_sb = tok.tile([1, TB], F32, name="e_sb", tag="e_sb")
                nc.scalar.activation(e_sb[:], pAt[0:1, :], AF.Exp, bias=b1[0:1, 7:8])

                pBC = ps.tile([128, TB], F32, name="pBC", tag="ps")
                nc.tensor.matmul(pBC[:], lhsT=ones1[:], rhs=e_sb[:], start=True, stop=True)
                ebc = tok.tile([128, TB], F32, name="ebc", tag="ebc")
                nc.any.tensor_copy(ebc[:], pBC[:])

                EV1 = tok.tile([128, TB], F32, name="EV1", tag="EV1")
                nc.vector.tensor_mul(EV1[:], pEC[:], ebc[:])
                EV2 = tok.tile([33, TB], F32, name="EV2", tag="EV2")
                nc.vector.memset(EV2[:], 0.0)
                nc.vector.tensor_mul(EV2[0:22, :], pECl[:], ebc[0:22, :])
                nc.any.tensor_copy(EV2[32:33, :], e_sb[:])

                C1 = tok.tile([128, TB], F32, name="C1", tag="c1")
                init1 = 0.0 if prevC1 is None else prevC1[:, prevTB - 1:prevTB]
                nc.vector.tensor_tensor_scan(C1[:], EV1[:], EV1[:], init1,
                                             op0=OP.add, op1=OP.bypass)
                C2 = tok.tile([33, TB], F32, name="C2", tag="c2")
                init2 = 0.0 if prevC2 is None else prevC2[:, prevTB - 1:prevTB]
                nc.vector.tensor_tensor_scan(C2[:], EV2[:], EV2[:], init2,
                                             op0=OP.add, op1=OP.bypass)
                prevC1, prevC2, prevTB = C1, C2, TB

                packed = tok.tile([54, TB], BF16, name="packed", tag="packed")
                nc.any.tensor_copy(packed[0:22, :], pL[32:54, :])
                nc.any.tensor_copy(packed[32:54, :], pL[64:86, :])

                A1 = tok.tile([128, TB], BF16, name="A1", tag="A1")
                nc.any.tensor_copy(A1[:], pA[:])
                B1 = tok.tile([128, TB], BF16, name="B1", tag="B1")
                nc.any.tensor_copy(B1[:], pB[:])

                nj = TB // 128
                afull = tok.tile([128, nj, 150], BF16, name="afull", tag="afull")
                bfull = tok.tile([128, nj, 150], BF16, name="bfull", tag="bfull")
                cfull = tok.tile([128, nj, 151], F32, name="cfull", tag="cfull")
                for j in range(nj):
                    js = j * 128

                    def tr(src_ap, name, dt, idn, psh=[128, 128]):
                        pt = ps.tile(psh, dt, name=f"pt_{name}", tag="ps")
                        kp = src_ap.shape[0]
                        nc.tensor.transpose(pt[:], src_ap, idn[0:kp, 0:kp])
                        return pt

                    pta = tr(A1[:, js:js + 128], "a1", BF16, identb)
                    ptb = tr(B1[:, js:js + 128], "b1", BF16, identb)
                    ptp = tr(packed[:, js:js + 128], "pk", BF16, identb, [128, 54])
                    ptc = tr(C1[:, js:js + 128], "c1", F32, ident)
                    ptc2 = tr(C2[:, js:js + 128], "c2", F32, ident, [128, 33])

                    nc.any.tensor_copy(afull[:, j, 0:128], pta[:])
                    nc.any.tensor_copy(afull[:, j, 128:150], ptp[:, 0:22])
                    nc.any.tensor_copy(bfull[:, j, 0:128], ptb[:])
                    nc.any.tensor_copy(bfull[:, j, 128:150], ptp[:, 32:54])
                    nc.any.tensor_copy(cfull[:, j, 0:128], ptc[:])
                    nc.any.tensor_copy(cfull[:, j, 128:150], ptc2[:, 0:22])
                    nc.any.tensor_copy(cfull[:, j, 150:151], ptc2[:, 32:33])

                nc.scalar.dma_start(
                    w1[t0:t0 + TB, 0:150].rearrange("(j p) c -> p j c", p=128),
                    afull[:])
                nc.sync.dma_start(
                    wB[t0:t0 + TB, :].rearrange("(j p) c -> p j c", p=128),
                    bfull[:])
                nc.scalar.dma_start(
                    w1[t0 + 1:t0 + TB + 1, 150:452].bitcast(F32)
                    .rearrange("(j p) c -> p j c", p=128),
                    cfull[:])
                if bi <= 6:
                    # duplicate early blocks into the late (complete) tables so
                    # group-B span tiles can reference any row
                    nc.sync.dma_start(
                        tabL[t0:t0 + TB, 0:150].rearrange("(j p) c -> p j c", p=128),
                        afull[:])
                    nc.scalar.dma_start(
                        tabBL[t0:t0 + TB, :].rearrange("(j p) c -> p j c", p=128),
                        bfull[:])
                    nc.sync.dma_start(
                        tabL[t0 + 1:t0 + TB + 1, 150:452].bitcast(F32)
                        .rearrange("(j p) c -> p j c", p=128),
                        cfull[:])
                if bi >= 6:
                    lo = (bi - 6) * 7
                    hi = min(NTA, lo + 7) if bi < 12 else NTA
                    emit_span_tiles(range(lo, hi), tabE, tabBE)

            emit_span_tiles(range(NTA, NT_S), tabL, tabBL)

    nc.compile()
    return nc


def _prep_shared(inputs):
    """Host-side weight packing (pure layout prep, shared by all cores)."""
    f32 = lambda x: np.ascontiguousarray(np.asarray(x), dtype=np.float32)
    aw1, ab1 = f32(inputs["aw1"]), f32(inputs["ab1"])
    aw2, ab2 = f32(inputs["aw2"]), f32(inputs["ab2"])
    aw3, ab3 = f32(inputs["aw3"]), f32(inputs["ab3"])
    sw1, sb1 = f32(inputs["sw1"]), f32(inputs["sb1"])
    sw2, sb2 = f32(inputs["sw2"]), f32(inputs["sb2"])
    sw3, sb3 = f32(inputs["sw3"]), f32(inputs["sb3"])
    wt = f32(inputs["width_table"])

    sw1a, sw1b, sw1p, sw1w = sw1[0:400], sw1[400:800], sw1[800:1150], sw1[1150:1170]

    wl = np.zeros((DS, 96), np.float32)
    wl[:, 0:22] = aw1[:, 128:150]
    wl[:, 32:54] = sw1a[:, 128:150]
    wl[:, 64:86] = sw1b[:, 128:150]

    out = {}
    wk4 = np.concatenate([aw1[:, 0:128], sw1a[:, 0:128], sw1b[:, 0:128], wl], axis=1)
    for i, (k0, k1) in enumerate(K400):
        out[f"wk4_{i}"] = np.ascontiguousarray(wk4[k0:k1]).astype(BF16NP)
    for i, (k0, k1) in enumerate(K350):
        out[f"wk3_{i}"] = np.ascontiguousarray(sw1p[k0:k1]).astype(BF16NP)
    wk1 = np.concatenate([aw2, sw2, aw3, sw3], axis=1)
    for i, (k0, k1) in enumerate(K150):
        out[f"wk1_{i}"] = np.ascontiguousarray(wk1[k0:k1]).astype(BF16NP)

    # width-bin difference table with sb1 folded in (widths are 1..10 -> bin 1..5)
    Wmb = wt @ sw1w  # [9, 150]
    dtab = np.zeros((5, H), np.float32)
    dtab[0] = Wmb[1] + sb1
    for jj in range(1, 5):
        dtab[jj] = Wmb[jj + 1] - Wmb[jj]
    out["dtab"] = dtab.astype(BF16NP)

    b1p = np.zeros((128, 8), np.float32)
    b1p[:, 0] = ab1[0:128]
    b1p[0:22, 1] = ab1[128:150]
    b1p[:, 2] = ab2[0:128]
    b1p[0:22, 3] = ab2[128:150]
    b1p[:, 4] = sb2[0:128]
    b1p[0:22, 5] = sb2[128:150]
    b1p[0, 6] = sb3[0]
    b1p[0, 7] = ab3[0]
    out["bias1"] = b1p
    return out


def prepare_in_maps(inputs):
    """Host-side sharding: returns (in_maps, sels) — per-core input dicts and
    the original span indices each core's padded slots map back to."""
    states = np.asarray(inputs["states"], dtype=np.float32)
    embeds = np.asarray(inputs["embeds"], dtype=np.float32)
    starts = np.asarray(inputs["span_starts"]).astype(np.int64)
    widths = np.asarray(inputs["span_widths"]).astype(np.int64)

    shared = _prep_shared(inputs)

    bucket = np.minimum(starts // TPC, NCORES - 1)
    order = np.argsort(bucket, kind="stable")
    counts = np.bincount(bucket, minlength=NCORES)
    assert counts.max() <= SMAX, f"span bucket overflow: {counts.max()} > {SMAX}"
    offs = np.zeros(NCORES + 1, np.int64)
    offs[1:] = np.cumsum(counts)

    mh_full = (widths[None, :] >= BINS5[:, None]).astype(np.float32)  # [5, S]

    in_maps = []
    sels = []
    for cix in range(NCORES):
        t0 = cix * TPC
        tl = min(T, t0 + TPC + W_MAX - 1) - t0
        seT = np.zeros((896, TL_PAD), BF16NP)
        seT[0:DS, :tl] = states[t0:t0 + tl].T.astype(BF16NP)
        seT[512:512 + DE, :tl] = embeds[t0:t0 + tl].T.astype(BF16NP)

        sel = order[offs[cix]:offs[cix + 1]]
        lsr = (starts[sel] - t0).astype(np.int32)
        ler = lsr + widths[sel].astype(np.int32) - 1
        # tiles 0..NTA-1 gather the early tables (rows < SPLIT): fill them
        # with spans whose end stays below the boundary, the rest go late
        capA = NTA * 128
        early_ix = np.nonzero(ler <= SPLIT - 1)[0]
        ia = early_ix[:capA]
        mask_rest = np.ones(len(sel), bool)
        mask_rest[ia] = False
        ib = np.nonzero(mask_rest)[0]
        assert len(ib) <= (NT_S - NTA) * 128, f"late overflow: {len(ib)}"
        slot_orig = np.full(SMAX, -1, np.int64)
        ls = np.zeros(SMAX, np.int32)
        le = np.zeros(SMAX, np.int32)
        slot_orig[:len(ia)] = sel[ia]
        ls[:len(ia)] = lsr[ia]
        le[:len(ia)] = ler[ia]
        ofB = capA
        slot_orig[ofB:ofB + len(ib)] = sel[ib]
        ls[ofB:ofB + len(ib)] = lsr[ib]
        le[ofB:ofB + len(ib)] = ler[ib]
        sels.append(slot_orig)
        mh = np.zeros((5, SMAX), BF16NP)
        mh[:, :len(ia)] = mh_full[:, sel[ia]].astype(BF16NP)
        mh[:, ofB:ofB + len(ib)] = mh_full[:, sel[ib]].astype(BF16NP)

        pm = lambda a: np.ascontiguousarray(a.reshape(NT_S, 128).T)
        # combined (start, end+1) index pairs, interleaved per span tile:
        # scomb[:, 2k] = starts of tile k, scomb[:, 2k+1] = ends+1 of tile k
        sc = np.empty((128, 2 * NT_S), np.int32)
        sc[:, 0::2] = pm(ls)
        sc[:, 1::2] = pm(le + 1)
        in_maps.append({
            "seT": seT,
            "scomb": np.ascontiguousarray(sc), "ends": pm(le),
            "mh": mh,
            **shared,
        })
    return in_maps, sels


def kernel(**inputs) -> np.ndarray:
    in_maps, sels = prepare_in_maps(inputs)

    if "nc" not in _PROGRAM_CACHE:
        _PROGRAM_CACHE["nc"] = _build_program()
    nc = _PROGRAM_CACHE["nc"]

    from concourse.bass_utils import run_bass_kernel_spmd
    res = run_bass_kernel_spmd(nc, in_maps, core_ids=list(range(NCORES)))
    _PROGRAM_CACHE["last_res"] = res  # exec_time_ns etc, for the test harness

    out = np.zeros(S, np.float32)
    for cix in range(NCORES):
        slot_orig = sels[cix]
        m = slot_orig >= 0
        vals = np.asarray(res.results[cix]["scores"]).reshape(-1)
        out[slot_orig[m]] = vals[m]
    return out



# revision 9
# speedup vs baseline: 1.0672x; 1.0672x over previous
"""MentionScore fused Bass kernel for 8 Trainium2 NeuronCores.

Strategy (self-contained, hardcoded for the nn_MentionScore problem):
  - Spans are bucketed by start//6250 -> one bucket per core; each core only
    needs its 6250-token slice (+9 halo), so states/embeds are sharded with
    no collectives.
  - Token phase (feature-major): per-token attention-logit MLP -> e = exp(a);
    the span MLP's first layer is decomposed through the gathers:
        g @ sw1 = A[start] + B[end] + (pooled @ sw1_p) + width-term
    with A = states@sw1[0:400], B = states@sw1[400:800], and per-token
    ec = e * (embeds@sw1[800:1150]).  A/B/ec(+e) are transposed to
    token-major tables resident in SBUF (no DRAM round trip).
  - Span phase: NO indirect DMA.  Spans are sorted by start so each tile of
    128 spans touches a narrow window of 1-3 aligned 128-token blocks; the
    per-span row selection is done with host-built one-hot / range masks as
    PE matmuls against the SBUF tables:
        A[start] = onehotS^T @ TA,  B[end] = onehotE^T @ TB,
        sum_{t in span} [ec|e][t] = range^T @ TEC.
    pooled term = ecsum/esum; width-bin embedding via a 5-column multi-hot
    matmul against a host-folded difference table.
  - The tile->window mapping is static (shared by all 8 SPMD cores): windows
    are the union of the 8 cores' per-tile token ranges, computed on host at
    build time; the program is cached per metadata tuple.
"""

import ml_dtypes
import numpy as np

BF16NP = ml_dtypes.bfloat16

# ---- problem constants (hardcoded per contract) ----
T, S = 50000, 100000
DS, DE, H, DW = 400, 350, 150, 20
W_MAX = 10
BINS5 = np.array([1, 2, 3, 4, 8], np.int64)
NCORES = 8
TPC = T // NCORES            # 6250 tokens per core bucket
TL_PAD = 6272                # 49 * 128 padded local tokens (6250 + 9 halo -> 6259)
NBLK = 49                    # 128-token blocks per core
TOK_BLOCKS = [(i * 512, 512) for i in range(12)] + [(6144, 128)]
K400 = [(0, 128), (128, 256), (256, 384), (384, 400)]
K350 = [(0, 128), (128, 256), (256, 350)]
K150 = [(0, 128), (128, 150)]

_PROGRAM_CACHE = {}


def _span_meta(starts, widths):
    """Per-core sorted span order + static (shared) tile window metadata.

    Returns (NT, tiles, percore) where tiles[k] = (slo, shi, elo, ehi, rlo,
    rhi, moff) block ranges for the start-onehot, end-onehot and range masks
    plus the tile's chunk-column offset, and percore[c] = (sel, ls, le) the
    original span indices and local start/end arrays padded to NT*128.
    """
    bucket = np.minimum(starts // TPC, NCORES - 1)
    percore = []
    counts = []
    for c in range(NCORES):
        sel = np.nonzero(bucket == c)[0]
        ls = (starts[sel] - c * TPC).astype(np.int32)
        order = np.argsort(ls, kind="stable")
        sel = sel[order]
        ls = ls[order]
        le = ls + widths[sel].astype(np.int32) - 1
        counts.append(len(sel))
        percore.append((sel, ls, le))
    NT = (max(counts) + 127) // 128
    # pad with the core's last span replicated (outputs discarded via sel<0)
    padded = []
    for sel, ls, le in percore:
        n = len(sel)
        pad = NT * 128 - n
        selp = np.concatenate([sel, np.full(pad, -1, np.int64)])
        lsp = np.concatenate([ls, np.full(pad, ls[-1], np.int32)])
        lep = np.concatenate([le, np.full(pad, le[-1], np.int32)])
        padded.append((selp, lsp, lep))
    tiles = []
    moff = 0
    for k in range(NT):
        slo = ehi = rlo = None
        a, b = k * 128, (k + 1) * 128
        slo = min(p[1][a] for p in padded) // 128
        shi = max(p[1][a:b].max() for p in padded) // 128
        elo = min(p[2][a:b].min() for p in padded) // 128
        ehi = max(p[2][a:b].max() for p in padded) // 128
        rlo, rhi = slo, ehi
        nch = (shi - slo + 1) + (ehi - elo + 1) + (rhi - rlo + 1)
        tiles.append((int(slo), int(shi), int(elo), int(ehi),
                      int(rlo), int(rhi), moff))
        moff += nch
    return NT, tuple(tiles), padded, moff


def _build_program(NT, tiles, total_chunks):
    import concourse.bacc as bacc
    import concourse.bass as bass
    import concourse.mybir as mybir
    import concourse.tile as tile
    from concourse.masks import make_identity

    F32 = mybir.dt.float32
    BF16 = mybir.dt.bfloat16
    AF = mybir.ActivationFunctionType
    OP = mybir.AluOpType

    SMAX = NT * 128

    nc = bacc.Bacc("TRN2", num_devices=NCORES)

    # ---- I/O ----
    # packed [states.T (400, pad to 512) | embeds.T (350, pad to 384)] = 896 rows
    seTd = nc.dram_tensor("seT", [896, TL_PAD], BF16, kind="ExternalInput")
    wk4d = [nc.dram_tensor(f"wk4_{i}", [k1 - k0, 480], BF16, kind="ExternalInput")
            for i, (k0, k1) in enumerate(K400)]
    wk3d = [nc.dram_tensor(f"wk3_{i}", [k1 - k0, 150], BF16, kind="ExternalInput")
            for i, (k0, k1) in enumerate(K350)]
    wk1d = [nc.dram_tensor(f"wk1_{i}", [k1 - k0, 302], BF16, kind="ExternalInput")
            for i, (k0, k1) in enumerate(K150)]
    dtabd = nc.dram_tensor("dtab", [5, H], BF16, kind="ExternalInput")
    b1d = nc.dram_tensor("bias1", [128, 8], F32, kind="ExternalInput")
    mhd = nc.dram_tensor("mh", [5, SMAX], BF16, kind="ExternalInput")
    masksd = nc.dram_tensor("masks", [128, total_chunks * 128], BF16,
                            kind="ExternalInput")
    scoresd = nc.dram_tensor("scores", [1, SMAX], F32, kind="ExternalOutput")

    with tile.TileContext(nc) as tc:
        with (
            tc.tile_pool(name="wpool", bufs=1) as wp,
            tc.tile_pool(name="tok", bufs=3) as tok,
            tc.tile_pool(name="span", bufs=4) as sp,
            tc.tile_pool(name="ps", bufs=7, space="PSUM") as ps,
            tc.tile_pool(name="psc", bufs=1, space="PSUM") as psc,
        ):
            # ---- resident weights / constants ----
            def wload(src, shape, name, dt=F32):
                t = wp.tile(shape, dt, name=name)
                nc.sync.dma_start(t[:], src)
                return t

            wk4 = [wload(wk4d[i][:, :], [k1 - k0, 480], f"wk4s_{i}", BF16) for i, (k0, k1) in enumerate(K400)]
            wk3 = [wload(wk3d[i][:, :], [k1 - k0, 150], f"wk3s_{i}", BF16) for i, (k0, k1) in enumerate(K350)]
            wk1 = [wload(wk1d[i][:, :], [k1 - k0, 302], f"wk1s_{i}", BF16) for i, (k0, k1) in enumerate(K150)]
            w_dt = wload(dtabd[:, :], [5, H], "wdt", BF16)
            b1 = wload(b1d[:, :], [128, 8], "b1")

            w_aw1 = [w[:, 0:128] for w in wk4]
            w_sa = [w[:, 128:256] for w in wk4]
            w_sb = [w[:, 256:384] for w in wk4]
            w_l4 = [w[:, 384:480] for w in wk4]
            w_pm = [w[:, 0:128] for w in wk3]
            w_pl = [w[:, 128:150] for w in wk3]
            w_a2m = [w[:, 0:128] for w in wk1]
            w_a2l = [w[:, 128:150] for w in wk1]
            w_s2m = [w[:, 150:278] for w in wk1]
            w_s2l = [w[:, 278:300] for w in wk1]
            w_a3 = [w[:, 300:301] for w in wk1]
            w_s3 = [w[:, 301:302] for w in wk1]

            mhs_sb = wp.tile([5, SMAX], BF16, name="mhs_sb")
            nc.sync.dma_start(mhs_sb[:], mhd[:, :])

            ones1 = wp.tile([128, 128], BF16, name="ones1")
            nc.vector.memset(ones1[:], 1.0)
            identb = wp.tile([128, 128], BF16, name="identb")
            make_identity(nc, identb[:])

            # resident token-major tables (bf16): TA/TB rows -> A/B, TEC
            # rows -> [ec(150) | e(1)]
            TA = wp.tile([128, NBLK, H], BF16, name="TA")
            TBt = wp.tile([128, NBLK, H], BF16, name="TBt")
            TEC = wp.tile([128, NBLK, H + 1], BF16, name="TEC")

            # ================= span phase =================
            scstate = {"pSC": None, "scs": None}

            def span_tile(k):
                slo, shi, elo, ehi, rlo, rhi, moff = tiles[k]
                nch = (shi - slo + 1) + (ehi - elo + 1) + (rhi - rlo + 1)
                mt = sp.tile([128, nch * 128], BF16, name="mt", tag="mt")
                nc.sync.dma_start(
                    mt[:], masksd[:, moff * 128:(moff + nch) * 128])

                # psw accumulates width-term + A[start] + B[end] (+ pooled
                # later); psEC accumulates [sum ec | sum e] over each span
                psw = ps.tile([128, H], F32, name="psw", tag="ps")
                nc.tensor.matmul(psw[:], lhsT=mhs_sb[:, k * 128:(k + 1) * 128],
                                 rhs=w_dt[:], start=True, stop=False)
                ci = 0
                for b in range(slo, shi + 1):
                    nc.tensor.matmul(psw[:], lhsT=mt[:, ci * 128:(ci + 1) * 128],
                                     rhs=TA[:, b, :], start=False, stop=False)
                    ci += 1
                for b in range(elo, ehi + 1):
                    nc.tensor.matmul(psw[:], lhsT=mt[:, ci * 128:(ci + 1) * 128],
                                     rhs=TBt[:, b, :], start=False, stop=False)
                    ci += 1
                psEC = ps.tile([128, H + 1], F32, name="psEC", tag="ps")
                for i, b in enumerate(range(rlo, rhi + 1)):
                    nc.tensor.matmul(psEC[:], lhsT=mt[:, ci * 128:(ci + 1) * 128],
                                     rhs=TEC[:, b, :], start=(i == 0),
                                     stop=(b == rhi))
                    ci += 1

                rec = sp.tile([128, 1], F32, name="rec", tag="rec")
                nc.vector.reciprocal(rec[:], psEC[:, H:H + 1])
                t1s = sp.tile([128, H], BF16, name="t1s", tag="t1s")
                nc.scalar.activation(t1s[:], psEC[:, 0:H], AF.Copy,
                                     scale=rec[:, 0:1])
                nc.tensor.matmul(psw[:], lhsT=identb[:], rhs=t1s[:],
                                 start=False, stop=True)
                h1s = sp.tile([128, H], BF16, name="h1s", tag="h1s")
                nc.vector.tensor_scalar(h1s[:], psw[:], 0.0, None, op0=OP.max)

                pT = ps.tile([128, 256], BF16, name="pT", tag="ps")
                nc.tensor.transpose(pT[:, 0:128], h1s[:, 0:128], identb[:])
                nc.tensor.transpose(pT[0:22, 128:256], h1s[:, 128:150], identb[:])
                h1t1 = sp.tile([128, 128], BF16, name="h1t1", tag="h1t1")
                nc.any.tensor_copy(h1t1[:], pT[:, 0:128])
                h1t2 = sp.tile([22, 128], BF16, name="h1t2", tag="h1t2")
                nc.any.tensor_copy(h1t2[:], pT[0:22, 128:256])

                pH2 = ps.tile([128, 256], F32, name="pH2", tag="ps")
                nc.tensor.matmul(pH2[:, 0:128], lhsT=w_s2m[0], rhs=h1t1[:], start=True, stop=False)
                nc.tensor.matmul(pH2[:, 0:128], lhsT=w_s2m[1], rhs=h1t2[:], start=False, stop=True)
                nc.tensor.matmul(pH2[0:22, 128:256], lhsT=w_s2l[0], rhs=h1t1[:], start=True, stop=False)
                nc.tensor.matmul(pH2[0:22, 128:256], lhsT=w_s2l[1], rhs=h1t2[:], start=False, stop=True)

                h2t1 = sp.tile([128, 128], BF16, name="h2t1", tag="h2t1")
                nc.scalar.activation(h2t1[:], pH2[:, 0:128], AF.Relu, bias=b1[0:128, 4:5])
                h2t2 = sp.tile([22, 128], BF16, name="h2t2", tag="h2t2")
                nc.vector.tensor_scalar(h2t2[:], pH2[0:22, 128:256],
                                        b1[0:22, 5:6], 0.0, op0=OP.add,
                                        op1=OP.max)

                j = k % 4
                if j == 0:
                    scstate["pSC"] = psc.tile([1, 512], F32, name="pSC", tag="sc")
                pSC = scstate["pSC"]
                sl = pSC[0:1, j * 128:(j + 1) * 128]
                nc.tensor.matmul(sl, lhsT=w_s3[0], rhs=h2t1[:], start=True, stop=False)
                nc.tensor.matmul(sl, lhsT=w_s3[1], rhs=h2t2[:], start=False, stop=True)

                g = k // 4
                if j == 3 or k == NT - 1:
                    width = (j + 1) * 128
                    if g % 4 == 0:
                        scstate["scs"] = sp.tile([1, 2048], F32, name="scs", tag="scs")
                    scs = scstate["scs"]
                    off = (g % 4) * 512
                    nc.vector.tensor_scalar(scs[0:1, off:off + width],
                                            pSC[0:1, 0:width],
                                            b1[0:1, 6:7], None, op0=OP.add)
                    if g % 4 == 3 or k == NT - 1:
                        lo = (g // 4) * 2048
                        w = off + width
                        nc.sync.dma_start(scoresd[0:1, lo:lo + w], scs[0:1, 0:w])

            # ================= token phase =================
            emitted = 0

            def emit_ready(max_blk):
                nonlocal emitted
                while emitted < NT and tiles[emitted][5] <= max_blk:
                    span_tile(emitted)
                    emitted += 1

            for bi, (t0, TB) in enumerate(TOK_BLOCKS):
                seb = tok.tile([128, 7, TB], BF16, name="seb", tag="seb")
                nc.sync.dma_start(
                    seb[:], seTd[:, t0:t0 + TB].rearrange("(j p) t -> p j t", p=128))
                st = [seb[0:128, 0, :], seb[0:128, 1, :], seb[0:128, 2, :],
                      seb[0:16, 3, :]]
                et = [seb[0:128, 4, :], seb[0:128, 5, :], seb[0:94, 6, :]]

                def mm_group(shape, lhs_list, rhs_list, name):
                    p = ps.tile(shape, F32, name=name, tag="ps")
                    n = len(lhs_list)
                    for i in range(n):
                        nc.tensor.matmul(p[:], lhsT=lhs_list[i], rhs=rhs_list[i],
                                         start=(i == 0), stop=(i == n - 1))
                    return p

                pH1 = mm_group([128, TB], w_aw1, st, "pH1")
                pA = mm_group([128, TB], w_sa, st, "pA")
                pB = mm_group([128, TB], w_sb, st, "pB")
                pL = mm_group([96, TB], w_l4, st, "pL")
                pEC = mm_group([128, TB], w_pm, et, "pEC")
                pECl = mm_group([22, TB], w_pl, et, "pECl")

                h1a = tok.tile([128, TB], BF16, name="h1a", tag="h1a")
                nc.scalar.activation(h1a[:], pH1[:], AF.Relu, bias=b1[0:128, 0:1])
                h1b = tok.tile([22, TB], BF16, name="h1b", tag="h1b")
                nc.scalar.activation(h1b[:], pL[0:22, :], AF.Relu, bias=b1[0:22, 1:2])

                pH2 = mm_group([128, TB], w_a2m, [h1a[:], h1b[:]], "pH2")
                pH2l = mm_group([22, TB], w_a2l, [h1a[:], h1b[:]], "pH2l")
                h2a = tok.tile([128, TB], BF16, name="h2a", tag="h2a")
                nc.scalar.activation(h2a[:], pH2[:], AF.Relu, bias=b1[0:128, 2:3])
                h2b = tok.tile([22, TB], BF16, name="h2b", tag="h2b")
                nc.scalar.activation(h2b[:], pH2l[:], AF.Relu, bias=b1[0:22, 3:4])

                pAt = mm_group([1, TB], w_a3, [h2a[:], h2b[:]], "pAt")
                # tails = [ecT 0:22 | saT 32:54 | sbT 64:86] (32-aligned rows so
                # the transposes satisfy the PE base-partition rule); e separate
                tails = tok.tile([87, TB], BF16, name="tails", tag="tails")
                e_sb = tok.tile([1, TB], BF16, name="e_sb", tag="e_sb")
                nc.scalar.activation(e_sb[:], pAt[0:1, :], AF.Exp,
                                     bias=b1[0:1, 7:8])

                pBC = ps.tile([128, TB], F32, name="pBC", tag="ps")
                nc.tensor.matmul(pBC[:], lhsT=ones1[0:1, :],
                                 rhs=e_sb[:], start=True, stop=True)
                ebc = tok.tile([128, TB], BF16, name="ebc", tag="ebc")
                nc.any.tensor_copy(ebc[:], pBC[:])

                A1 = tok.tile([128, TB], BF16, name="A1", tag="A1")
                nc.any.tensor_copy(A1[:], pA[:])
                B1 = tok.tile([128, TB], BF16, name="B1", tag="B1")
                nc.any.tensor_copy(B1[:], pB[:])
                EV1 = tok.tile([128, TB], BF16, name="EV1", tag="EV1")
                nc.vector.tensor_mul(EV1[:], pEC[:], ebc[:])
                nc.any.tensor_copy(tails[32:54, :], pL[32:54, :])
                nc.any.tensor_copy(tails[64:86, :], pL[64:86, :])
                nc.vector.tensor_mul(tails[0:22, :], pECl[:], ebc[0:22, :])

                nj = TB // 128
                for j in range(nj):
                    js = j * 128
                    blk = 4 * bi + j
                    ptA = ps.tile([128, 152], BF16, name="ptA", tag="ps")
                    nc.tensor.transpose(ptA[:, 0:128], A1[:, js:js + 128], identb[:])
                    nc.tensor.transpose(ptA[:, 128:150], tails[32:54, js:js + 128],
                                        identb[32:54, 32:54])
                    ptB = ps.tile([128, 152], BF16, name="ptB", tag="ps")
                    nc.tensor.transpose(ptB[:, 0:128], B1[:, js:js + 128], identb[:])
                    nc.tensor.transpose(ptB[:, 128:150], tails[64:86, js:js + 128],
                                        identb[64:86, 64:86])
                    ptE = ps.tile([128, 152], BF16, name="ptE", tag="ps")
                    nc.tensor.transpose(ptE[:, 0:128], EV1[:, js:js + 128], identb[:])
                    nc.tensor.transpose(ptE[:, 128:150], tails[0:22, js:js + 128],
                                        identb[0:22, 0:22])
                    nc.tensor.transpose(ptE[:, 150:151], e_sb[0:1, js:js + 128],
                                        identb[0:1, 0:1])
                    nc.any.tensor_copy(TA[:, blk, :], ptA[:, 0:150])
                    nc.any.tensor_copy(TBt[:, blk, :], ptB[:, 0:150])
                    nc.any.tensor_copy(TEC[:, blk, :], ptE[:, 0:151])

                emit_ready(4 * bi + nj - 1)

            emit_ready(NBLK)

    nc.compile()
    return nc


def _prep_shared(inputs):
    """Host-side weight packing (pure layout prep, shared by all cores)."""
    f32 = lambda x: np.ascontiguousarray(np.asarray(x), dtype=np.float32)
    aw1, ab1 = f32(inputs["aw1"]), f32(inputs["ab1"])
    aw2, ab2 = f32(inputs["aw2"]), f32(inputs["ab2"])
    aw3, ab3 = f32(inputs["aw3"]), f32(inputs["ab3"])
    sw1, sb1 = f32(inputs["sw1"]), f32(inputs["sb1"])
    sw2, sb2 = f32(inputs["sw2"]), f32(inputs["sb2"])
    sw3, sb3 = f32(inputs["sw3"]), f32(inputs["sb3"])
    wt = f32(inputs["width_table"])

    sw1a, sw1b, sw1p, sw1w = sw1[0:400], sw1[400:800], sw1[800:1150], sw1[1150:1170]

    # wl: [aw1 tail 0:22 | sw1a tail 32:54 | sw1b tail 64:86] (32-aligned)
    wl = np.zeros((DS, 96), np.float32)
    wl[:, 0:22] = aw1[:, 128:150]
    wl[:, 32:54] = sw1a[:, 128:150]
    wl[:, 64:86] = sw1b[:, 128:150]

    out = {}
    wk4 = np.concatenate([aw1[:, 0:128], sw1a[:, 0:128], sw1b[:, 0:128], wl], axis=1)
    for i, (k0, k1) in enumerate(K400):
        out[f"wk4_{i}"] = np.ascontiguousarray(wk4[k0:k1]).astype(BF16NP)
    for i, (k0, k1) in enumerate(K350):
        out[f"wk3_{i}"] = np.ascontiguousarray(sw1p[k0:k1]).astype(BF16NP)
    wk1 = np.concatenate([aw2, sw2, aw3, sw3], axis=1)
    for i, (k0, k1) in enumerate(K150):
        out[f"wk1_{i}"] = np.ascontiguousarray(wk1[k0:k1]).astype(BF16NP)

    # width-bin difference table with sb1 folded in (widths are 1..10 -> bin 1..5)
    Wmb = wt @ sw1w  # [9, 150]
    dtab = np.zeros((5, H), np.float32)
    dtab[0] = Wmb[1] + sb1
    for jj in range(1, 5):
        dtab[jj] = Wmb[jj + 1] - Wmb[jj]
    out["dtab"] = dtab.astype(BF16NP)

    b1p = np.zeros((128, 8), np.float32)
    b1p[:, 0] = ab1[0:128]
    b1p[0:22, 1] = ab1[128:150]
    b1p[:, 2] = ab2[0:128]
    b1p[0:22, 3] = ab2[128:150]
    b1p[:, 4] = sb2[0:128]
    b1p[0:22, 5] = sb2[128:150]
    b1p[0, 6] = sb3[0]
    b1p[0, 7] = ab3[0]
    out["bias1"] = b1p
    return out


def prepare_in_maps(inputs):
    """Host-side sharding: returns (in_maps, sels, meta)."""
    states = np.asarray(inputs["states"], dtype=np.float32)
    embeds = np.asarray(inputs["embeds"], dtype=np.float32)
    starts = np.asarray(inputs["span_starts"]).astype(np.int64)
    widths = np.asarray(inputs["span_widths"]).astype(np.int64)

    shared = _prep_shared(inputs)
    NT, tiles, padded, total_chunks = _span_meta(starts, widths)
    SMAX = NT * 128

    mh_full = (widths[None, :] >= BINS5[:, None]).astype(np.float32)  # [5, S]
    rows = np.arange(128, dtype=np.int32)

    in_maps = []
    sels = []
    for cix in range(NCORES):
        t0 = cix * TPC
        tl = min(T, t0 + TPC + W_MAX - 1) - t0
        seT = np.zeros((896, TL_PAD), BF16NP)
        seT[0:DS, :tl] = states[t0:t0 + tl].T.astype(BF16NP)
        seT[512:512 + DE, :tl] = embeds[t0:t0 + tl].T.astype(BF16NP)

        selp, ls, le = padded[cix]
        sels.append(selp)

        mh = np.zeros((5, SMAX), BF16NP)
        real = selp >= 0
        mh[:, real] = mh_full[:, selp[real]].astype(BF16NP)

        masks = np.zeros((128, total_chunks * 128), BF16NP)
        for k in range(NT):
            slo, shi, elo, ehi, rlo, rhi, moff = tiles[k]
            a = k * 128
            lsk = ls[a:a + 128]
            lek = le[a:a + 128]
            ci = moff
            for b in range(slo, shi + 1):
                tk = 128 * b + rows
                masks[:, ci * 128:(ci + 1) * 128] = \
                    (lsk[None, :] == tk[:, None]).astype(BF16NP)
                ci += 1
            for b in range(elo, ehi + 1):
                tk = 128 * b + rows
                masks[:, ci * 128:(ci + 1) * 128] = \
                    (lek[None, :] == tk[:, None]).astype(BF16NP)
                ci += 1
            for b in range(rlo, rhi + 1):
                tk = 128 * b + rows
                masks[:, ci * 128:(ci + 1) * 128] = \
                    ((lsk[None, :] <= tk[:, None]) &
                     (tk[:, None] <= lek[None, :])).astype(BF16NP)
                ci += 1

        in_maps.append({
            "seT": seT,
            "mh": mh,
            "masks": masks,
            **shared,
        })
    return in_maps, sels, (NT, tiles, total_chunks)


def kernel(**inputs) -> np.ndarray:
    in_maps, sels, meta = prepare_in_maps(inputs)
    NT, tiles, total_chunks = meta

    key = (NT, tiles, total_chunks)
    if key not in _PROGRAM_CACHE:
        _PROGRAM_CACHE[key] = _build_program(NT, tiles, total_chunks)
        _PROGRAM_CACHE["nc"] = _PROGRAM_CACHE[key]
    nc = _PROGRAM_CACHE[key]

    from concourse.bass_utils import run_bass_kernel_spmd
    res = run_bass_kernel_spmd(nc, in_maps, core_ids=list(range(NCORES)))
    _PROGRAM_CACHE["last_res"] = res  # exec_time_ns etc, for the test harness

    out = np.zeros(S, np.float32)
    for cix in range(NCORES):
        slot_orig = sels[cix]
        m = slot_orig >= 0
        vals = np.asarray(res.results[cix]["scores"]).reshape(-1)
        out[slot_orig[m]] = vals[m]
    return out


# revision 17
# speedup vs baseline: 2.2688x; 2.1259x over previous
"""MentionScore fused Bass kernel for 8 Trainium2 NeuronCores.

Strategy (self-contained, hardcoded for the nn_MentionScore problem):
  - Spans are bucketed by start//6250 -> one bucket per core; each core only
    needs its 6250-token slice (+9 halo), so states/embeds are sharded with
    no collectives.
  - Token phase (feature-major): per-token attention-logit MLP -> e = exp(a);
    the span MLP's first layer is decomposed through the gathers:
        g @ sw1 = A[start] + B[end] + (pooled @ sw1_p) + width-term
    with A = states@sw1[0:400], B = states@sw1[400:800], and per-token
    ec = e * (embeds@sw1[800:1150]).  A/B/ec(+e) are transposed to
    token-major tables resident in SBUF (no DRAM round trip).
  - Span phase: NO indirect DMA.  Spans are sorted by start so each tile of
    128 spans touches a narrow window of 1-3 aligned 128-token blocks; the
    per-span row selection is done with host-built one-hot / range masks as
    PE matmuls against the SBUF tables:
        A[start] = onehotS^T @ TA,  B[end] = onehotE^T @ TB,
        sum_{t in span} [ec|e][t] = range^T @ TEC.
    pooled term = ecsum/esum; width-bin embedding via a 5-column multi-hot
    matmul against a host-folded difference table.
  - The tile->window mapping is static (shared by all 8 SPMD cores): windows
    are the union of the 8 cores' per-tile token ranges, computed on host at
    build time; the program is cached per metadata tuple.
"""

import ml_dtypes
import numpy as np

BF16NP = ml_dtypes.bfloat16

# ---- problem constants (hardcoded per contract) ----
T, S = 50000, 100000
DS, DE, H, DW = 400, 350, 150, 20
W_MAX = 10
BINS5 = np.array([1, 2, 3, 4, 8], np.int64)
NCORES = 8
TPC = T // NCORES            # 6250 tokens per core bucket
TL_PAD = 6272                # 49 * 128 padded local tokens (6250 + 9 halo -> 6259)
NBLK = 49                    # 128-token blocks per core
TOK_BLOCKS = [(i * 512, 512) for i in range(12)] + [(6144, 128)]
K400 = [(0, 128), (128, 256), (256, 384), (384, 400)]
K350 = [(0, 128), (128, 256), (256, 350)]
K150 = [(0, 128), (128, 150)]

_PROGRAM_CACHE = {}


def _span_meta(starts, widths):
    """Per-core sorted span order + static (shared) tile window metadata.

    Returns (NT, tiles, percore) where tiles[k] = (slo, shi, elo, ehi, rlo,
    rhi, moff) block ranges for the start-onehot, end-onehot and range masks
    plus the tile's chunk-column offset, and percore[c] = (sel, ls, le) the
    original span indices and local start/end arrays padded to NT*128.
    """
    bucket = np.minimum(starts // TPC, NCORES - 1)
    percore = []
    counts = []
    for c in range(NCORES):
        sel = np.nonzero(bucket == c)[0]
        ls = (starts[sel] - c * TPC).astype(np.int32)
        order = np.argsort(ls, kind="stable")
        sel = sel[order]
        ls = ls[order]
        le = ls + widths[sel].astype(np.int32) - 1
        counts.append(len(sel))
        percore.append((sel, ls, le))
    NT = (max(counts) + 127) // 128
    # pad with the core's last span replicated (outputs discarded via sel<0)
    padded = []
    for sel, ls, le in percore:
        n = len(sel)
        pad = NT * 128 - n
        selp = np.concatenate([sel, np.full(pad, -1, np.int64)])
        lsp = np.concatenate([ls, np.full(pad, ls[-1], np.int32)])
        lep = np.concatenate([le, np.full(pad, le[-1], np.int32)])
        padded.append((selp, lsp, lep))
    tiles = []
    moff = 0
    for k in range(NT):
        slo = ehi = rlo = None
        a, b = k * 128, (k + 1) * 128
        slo = min(p[1][a] for p in padded) // 128
        shi = max(p[1][a:b].max() for p in padded) // 128
        elo = min(p[2][a:b].min() for p in padded) // 128
        ehi = max(p[2][a:b].max() for p in padded) // 128
        rlo, rhi = slo, ehi
        nch = (shi - slo + 1) + (ehi - elo + 1) + (rhi - rlo + 1)
        tiles.append((int(slo), int(shi), int(elo), int(ehi),
                      int(rlo), int(rhi), moff))
        moff += nch
    return NT, tuple(tiles), padded, moff


def _build_program(NT, tiles, total_chunks):
    import concourse.bacc as bacc
    import concourse.bass as bass
    import concourse.mybir as mybir
    import concourse.tile as tile
    from concourse.masks import make_identity

    F32 = mybir.dt.float32
    BF16 = mybir.dt.bfloat16
    AF = mybir.ActivationFunctionType
    OP = mybir.AluOpType

    SMAX = NT * 128

    nc = bacc.Bacc("TRN2", num_devices=NCORES)

    # ---- I/O ----
    # packed [states.T (400, pad to 512) | embeds.T (350, pad to 384)] = 896 rows
    seTd = nc.dram_tensor("seT", [896, TL_PAD], BF16, kind="ExternalInput")
    wk4d = [nc.dram_tensor(f"wk4_{i}", [k1 - k0, 480], BF16, kind="ExternalInput")
            for i, (k0, k1) in enumerate(K400)]
    wk3d = [nc.dram_tensor(f"wk3_{i}", [k1 - k0, 150], BF16, kind="ExternalInput")
            for i, (k0, k1) in enumerate(K350)]
    wk1d = [nc.dram_tensor(f"wk1_{i}", [k1 - k0, 302], BF16, kind="ExternalInput")
            for i, (k0, k1) in enumerate(K150)]
    dtabd = nc.dram_tensor("dtab", [5, H], BF16, kind="ExternalInput")
    b1d = nc.dram_tensor("bias1", [128, 8], F32, kind="ExternalInput")
    mhd = nc.dram_tensor("mh", [5, SMAX], BF16, kind="ExternalInput")
    masksd = nc.dram_tensor("masks", [128, total_chunks * 128], BF16,
                            kind="ExternalInput")
    scoresd = nc.dram_tensor("scores", [1, SMAX], F32, kind="ExternalOutput")

    with tile.TileContext(nc) as tc:
        with (
            tc.tile_pool(name="wpool", bufs=1) as wp,
            tc.tile_pool(name="tok", bufs=3) as tok,
            tc.tile_pool(name="span", bufs=6) as sp,
            tc.tile_pool(name="ps", bufs=4, space="PSUM") as ps,
            tc.tile_pool(name="ps1", bufs=2, space="PSUM") as ps1,
            tc.tile_pool(name="ps2", bufs=2, space="PSUM") as ps2,
        ):
            # ---- resident weights / constants ----
            def wload(src, shape, name, dt=F32):
                t = wp.tile(shape, dt, name=name)
                nc.sync.dma_start(t[:], src)
                return t

            wk4 = [wload(wk4d[i][:, :], [k1 - k0, 480], f"wk4s_{i}", BF16) for i, (k0, k1) in enumerate(K400)]
            wk3 = [wload(wk3d[i][:, :], [k1 - k0, 150], f"wk3s_{i}", BF16) for i, (k0, k1) in enumerate(K350)]
            wk1 = [wload(wk1d[i][:, :], [k1 - k0, 302], f"wk1s_{i}", BF16) for i, (k0, k1) in enumerate(K150)]
            # second copy of the 22-row weight tail parked at partitions 32:54
            # so span-phase tail matmuls can pair with base-32 rhs operands
            wk1b = wp.tile([54, 302], BF16, name="wk1b")
            nc.sync.dma_start(wk1b[32:54, :], wk1d[1][:, :])
            w_dt = wload(dtabd[:, :], [5, H], "wdt", BF16)
            b1 = wload(b1d[:, :], [128, 8], "b1")

            w_aw1 = [w[:, 0:128] for w in wk4]
            w_sa = [w[:, 128:256] for w in wk4]
            w_sb = [w[:, 256:384] for w in wk4]
            w_l4 = [w[:, 384:480] for w in wk4]
            w_pm = [w[:, 0:128] for w in wk3]
            w_pl = [w[:, 128:150] for w in wk3]
            w_a2m = [w[:, 0:128] for w in wk1]
            w_a2l = [w[:, 128:150] for w in wk1]
            w_s2m = [w[:, 150:278] for w in wk1]
            w_s2l = [w[:, 278:300] for w in wk1]
            w_a3 = [w[:, 300:301] for w in wk1]
            w_s3 = [w[:, 301:302] for w in wk1]

            mhs_sb = wp.tile([5, SMAX], BF16, name="mhs_sb")
            nc.sync.dma_start(mhs_sb[:], mhd[:, :])

            ones1 = wp.tile([128, 128], BF16, name="ones1")
            nc.vector.memset(ones1[:], 1.0)
            identb = wp.tile([128, 128], BF16, name="identb")
            make_identity(nc, identb[:])

            # resident token-major tables (bf16): TA/TB rows -> A/B, TEC
            # rows -> [ec(150) | e(1)]
            TA = wp.tile([128, NBLK, H], BF16, name="TA")
            TBt = wp.tile([128, NBLK, H], BF16, name="TBt")
            TEC = wp.tile([128, NBLK, H + 1], BF16, name="TEC")

            # ================= span phase =================
            # P1 bank layout (f32 cols): psw 0:150 | psEC 152:303 | pT(bf16
            # bitcast) 304:432.  P2: pH2 0:256 | pSC [0:1, 256:384].
            scstate = {"scs": None}

            def span_tile(k):
                slo, shi, elo, ehi, rlo, rhi, moff = tiles[k]
                nch = (shi - slo + 1) + (ehi - elo + 1) + (rhi - rlo + 1)
                mt = sp.tile([128, nch * 128], BF16, name="mt", tag="mt")
                nc.sync.dma_start(
                    mt[:], masksd[:, moff * 128:(moff + nch) * 128])

                P1 = ps1.tile([128, 432], F32, name="P1", tag="p1")
                psw = P1[:, 0:150]
                psEC = P1[:, 152:303]
                pT = P1[:, 304:432].bitcast(BF16)
                P2 = ps2.tile([128, 384], F32, name="P2", tag="p2")
                pH2 = P2[:, 0:256]
                pSC = P2[0:1, 256:384]

                # psw: width-term + A[start] + B[end] + pooled (via h1f later)
                nc.tensor.matmul(psw, lhsT=mhs_sb[:, k * 128:(k + 1) * 128],
                                 rhs=w_dt[:], start=True, stop=False)
                ci = 0
                for b in range(slo, shi + 1):
                    nc.tensor.matmul(psw, lhsT=mt[:, ci * 128:(ci + 1) * 128],
                                     rhs=TA[:, b, :], start=False, stop=False)
                    ci += 1
                for b in range(elo, ehi + 1):
                    nc.tensor.matmul(psw, lhsT=mt[:, ci * 128:(ci + 1) * 128],
                                     rhs=TBt[:, b, :], start=False,
                                     stop=(b == ehi))
                    ci += 1
                for i, b in enumerate(range(rlo, rhi + 1)):
                    nc.tensor.matmul(psEC, lhsT=mt[:, ci * 128:(ci + 1) * 128],
                                     rhs=TEC[:, b, :], start=(i == 0),
                                     stop=(b == rhi))
                    ci += 1

                rec = sp.tile([128, 1], F32, name="rec", tag="rec")
                nc.vector.reciprocal(rec[:], P1[:, 302:303])
                # pooled term = psEC * (1/esum), then + psw (relu deferred to
                # the post-transpose copy; only one PSUM operand per DVE op)
                t1s = sp.tile([128, H], F32, name="t1s", tag="t1s")
                nc.scalar.activation(t1s[:], P1[:, 152:302], AF.Copy,
                                     scale=rec[:, 0:1])
                h1f = sp.tile([128, H], BF16, name="h1f", tag="h1f")
                nc.vector.tensor_add(h1f[:], t1s[:], psw)

                nc.tensor.transpose(pT[:, 0:128], h1f[:, 0:128], identb[:])
                nc.tensor.transpose(pT[32:54, 128:256], h1f[:, 128:150], identb[:])
                # relu folded into the PSUM->SBUF copy of the transpose
                h1t = sp.tile([128, 256], BF16, name="h1t", tag="h1t")
                nc.vector.tensor_scalar(h1t[:], pT[:], 0.0, None, op0=OP.max)

                nc.tensor.matmul(pH2[:, 0:128], lhsT=w_s2m[0], rhs=h1t[:, 0:128], start=True, stop=False)
                nc.tensor.matmul(pH2[:, 0:128], lhsT=wk1b[32:54, 150:278], rhs=h1t[32:54, 128:256], start=False, stop=True)
                nc.tensor.matmul(pH2[32:54, 128:256], lhsT=w_s2l[0], rhs=h1t[:, 0:128], start=True, stop=False)
                nc.tensor.matmul(pH2[32:54, 128:256], lhsT=wk1b[32:54, 278:300], rhs=h1t[32:54, 128:256], start=False, stop=True)

                h2t1 = sp.tile([128, 128], BF16, name="h2t1", tag="h2t1")
                nc.scalar.activation(h2t1[:], pH2[:, 0:128], AF.Relu, bias=b1[0:128, 4:5])
                h2t2 = sp.tile([54, 128], BF16, name="h2t2", tag="h2t2")
                nc.vector.tensor_scalar(h2t2[32:54, :], pH2[32:54, 128:256],
                                        0.0, None, op0=OP.max)

                nc.tensor.matmul(pSC, lhsT=w_s3[0], rhs=h2t1[:], start=True, stop=False)
                nc.tensor.matmul(pSC, lhsT=wk1b[32:54, 301:302], rhs=h2t2[32:54, :], start=False, stop=True)

                if k % 16 == 0:
                    scstate["scs"] = sp.tile([1, 2048], F32, name="scs", tag="scs")
                scs = scstate["scs"]
                off = (k % 16) * 128
                nc.vector.tensor_scalar(scs[0:1, off:off + 128], pSC,
                                        b1[0:1, 6:7], None, op0=OP.add)
                if k % 16 == 15 or k == NT - 1:
                    lo = (k // 16) * 2048
                    w = off + 128
                    nc.sync.dma_start(scoresd[0:1, lo:lo + w], scs[0:1, 0:w])

            # ================= token phase =================
            emitted = 0

            def emit_ready(max_blk):
                nonlocal emitted
                while emitted < NT and tiles[emitted][5] <= max_blk:
                    span_tile(emitted)
                    emitted += 1

            for bi, (t0, TB) in enumerate(TOK_BLOCKS):
                seb = tok.tile([128, 7, TB], BF16, name="seb", tag="seb")
                nc.sync.dma_start(
                    seb[:], seTd[:, t0:t0 + TB].rearrange("(j p) t -> p j t", p=128))
                st = [seb[0:128, 0, :], seb[0:128, 1, :], seb[0:128, 2, :],
                      seb[0:16, 3, :]]
                et = [seb[0:128, 4, :], seb[0:128, 5, :], seb[0:94, 6, :]]

                def mm_group(shape, lhs_list, rhs_list, name):
                    p = ps.tile(shape, F32, name=name, tag="ps")
                    n = len(lhs_list)
                    for i in range(n):
                        nc.tensor.matmul(p[:], lhsT=lhs_list[i], rhs=rhs_list[i],
                                         start=(i == 0), stop=(i == n - 1))
                    return p

                pH1 = mm_group([128, TB], w_aw1, st, "pH1")
                pA = mm_group([128, TB], w_sa, st, "pA")
                pB = mm_group([128, TB], w_sb, st, "pB")
                pL = mm_group([96, TB], w_l4, st, "pL")
                pEC = mm_group([128, TB], w_pm, et, "pEC")
                pECl = mm_group([22, TB], w_pl, et, "pECl")

                h1a = tok.tile([128, TB], BF16, name="h1a", tag="h1a")
                nc.scalar.activation(h1a[:], pH1[:], AF.Relu, bias=b1[0:128, 0:1])
                h1b = tok.tile([22, TB], BF16, name="h1b", tag="h1b")
                nc.scalar.activation(h1b[:], pL[0:22, :], AF.Relu, bias=b1[0:22, 1:2])

                pH2 = mm_group([128, TB], w_a2m, [h1a[:], h1b[:]], "pH2")
                pH2l = mm_group([22, TB], w_a2l, [h1a[:], h1b[:]], "pH2l")
                h2a = tok.tile([128, TB], BF16, name="h2a", tag="h2a")
                nc.scalar.activation(h2a[:], pH2[:], AF.Relu, bias=b1[0:128, 2:3])
                h2b = tok.tile([22, TB], BF16, name="h2b", tag="h2b")
                nc.scalar.activation(h2b[:], pH2l[:], AF.Relu, bias=b1[0:22, 3:4])

                pAt = mm_group([1, TB], w_a3, [h2a[:], h2b[:]], "pAt")
                # tails = [ecT 0:22 | saT 32:54 | sbT 64:86] (32-aligned rows so
                # the transposes satisfy the PE base-partition rule); e separate
                tails = tok.tile([87, TB], BF16, name="tails", tag="tails")
                e_sb = tok.tile([1, TB], BF16, name="e_sb", tag="e_sb")
                nc.scalar.activation(e_sb[:], pAt[0:1, :], AF.Exp,
                                     bias=b1[0:1, 7:8])

                A1 = tok.tile([128, TB], BF16, name="A1", tag="A1")
                nc.vector.tensor_copy(A1[:], pA[:])
                B1 = tok.tile([128, TB], BF16, name="B1", tag="B1")
                nc.vector.tensor_copy(B1[:], pB[:])
                EC1 = tok.tile([128, TB], BF16, name="EC1", tag="EC1")
                nc.vector.tensor_copy(EC1[:], pEC[:])
                nc.scalar.copy(tails[32:54, :], pL[32:54, :])
                nc.scalar.copy(tails[64:86, :], pL[64:86, :])
                nc.vector.tensor_copy(tails[0:22, :], pECl[:])

                nj = TB // 128
                for j in range(nj):
                    js = j * 128
                    blk = 4 * bi + j
                    # one PSUM bank for all three transposed slabs:
                    # A 0:152 | B 152:304 | EC+e 304:456
                    pt = ps.tile([128, 456], BF16, name="pt", tag="ps")
                    nc.tensor.transpose(pt[:, 0:128], A1[:, js:js + 128], identb[:])
                    nc.tensor.transpose(pt[:, 128:150], tails[32:54, js:js + 128],
                                        identb[32:54, 32:54])
                    nc.tensor.transpose(pt[:, 152:280], B1[:, js:js + 128], identb[:])
                    nc.tensor.transpose(pt[:, 280:302], tails[64:86, js:js + 128],
                                        identb[64:86, 64:86])
                    nc.tensor.transpose(pt[:, 304:432], EC1[:, js:js + 128], identb[:])
                    nc.tensor.transpose(pt[:, 432:454], tails[0:22, js:js + 128],
                                        identb[0:22, 0:22])
                    nc.tensor.transpose(pt[:, 454:455], e_sb[0:1, js:js + 128],
                                        identb[0:1, 0:1])
                    nc.scalar.copy(TA[:, blk, :], pt[:, 0:150])
                    nc.scalar.copy(TBt[:, blk, :], pt[:, 152:302])
                    # e column first (f32 staging: tensor_scalar needs an f32
                    # scalar operand), then scale the ec columns by it
                    ecol = tok.tile([128, 1], F32, name="ecol", tag="ecol")
                    nc.vector.tensor_copy(ecol[:], pt[:, 454:455])
                    nc.vector.tensor_copy(TEC[:, blk, 150:151], pt[:, 454:455])
                    nc.vector.tensor_scalar(TEC[:, blk, 0:150], pt[:, 304:454],
                                            ecol[:, 0:1], None, op0=OP.mult)

                emit_ready(4 * bi + nj - 1)

            emit_ready(NBLK)

    nc.compile()
    return nc


def _prep_shared(inputs):
    """Host-side weight packing (pure layout prep, shared by all cores)."""
    f32 = lambda x: np.ascontiguousarray(np.asarray(x), dtype=np.float32)
    aw1, ab1 = f32(inputs["aw1"]), f32(inputs["ab1"])
    aw2, ab2 = f32(inputs["aw2"]), f32(inputs["ab2"])
    aw3, ab3 = f32(inputs["aw3"]), f32(inputs["ab3"])
    sw1, sb1 = f32(inputs["sw1"]), f32(inputs["sb1"])
    sw2, sb2 = f32(inputs["sw2"]), f32(inputs["sb2"])
    sw3, sb3 = f32(inputs["sw3"]), f32(inputs["sb3"])
    wt = f32(inputs["width_table"])

    sw1a, sw1b, sw1p, sw1w = sw1[0:400], sw1[400:800], sw1[800:1150], sw1[1150:1170]

    # wl: [aw1 tail 0:22 | sw1a tail 32:54 | sw1b tail 64:86] (32-aligned)
    wl = np.zeros((DS, 96), np.float32)
    wl[:, 0:22] = aw1[:, 128:150]
    wl[:, 32:54] = sw1a[:, 128:150]
    wl[:, 64:86] = sw1b[:, 128:150]

    out = {}
    wk4 = np.concatenate([aw1[:, 0:128], sw1a[:, 0:128], sw1b[:, 0:128], wl], axis=1)
    for i, (k0, k1) in enumerate(K400):
        out[f"wk4_{i}"] = np.ascontiguousarray(wk4[k0:k1]).astype(BF16NP)
    for i, (k0, k1) in enumerate(K350):
        out[f"wk3_{i}"] = np.ascontiguousarray(sw1p[k0:k1]).astype(BF16NP)
    wk1 = np.concatenate([aw2, sw2, aw3, sw3], axis=1)
    for i, (k0, k1) in enumerate(K150):
        out[f"wk1_{i}"] = np.ascontiguousarray(wk1[k0:k1]).astype(BF16NP)

    # width-bin difference table with sb1 folded in (widths are 1..10 -> bin 1..5)
    Wmb = wt @ sw1w  # [9, 150]
    dtab = np.zeros((5, H), np.float32)
    dtab[0] = Wmb[1] + sb1
    for jj in range(1, 5):
        dtab[jj] = Wmb[jj + 1] - Wmb[jj]
    out["dtab"] = dtab.astype(BF16NP)

    b1p = np.zeros((128, 8), np.float32)
    b1p[:, 0] = ab1[0:128]
    b1p[0:22, 1] = ab1[128:150]
    b1p[:, 2] = ab2[0:128]
    b1p[0:22, 3] = ab2[128:150]
    b1p[:, 4] = sb2[0:128]
    b1p[0:22, 5] = sb2[128:150]
    b1p[0, 6] = sb3[0]
    b1p[0, 7] = ab3[0]
    out["bias1"] = b1p
    return out


def prepare_in_maps(inputs):
    """Host-side sharding: returns (in_maps, sels, meta)."""
    states = np.asarray(inputs["states"], dtype=np.float32)
    embeds = np.asarray(inputs["embeds"], dtype=np.float32)
    starts = np.asarray(inputs["span_starts"]).astype(np.int64)
    widths = np.asarray(inputs["span_widths"]).astype(np.int64)

    shared = _prep_shared(inputs)
    NT, tiles, padded, total_chunks = _span_meta(starts, widths)
    SMAX = NT * 128

    mh_full = (widths[None, :] >= BINS5[:, None]).astype(np.float32)  # [5, S]
    rows = np.arange(128, dtype=np.int32)

    in_maps = []
    sels = []
    for cix in range(NCORES):
        t0 = cix * TPC
        tl = min(T, t0 + TPC + W_MAX - 1) - t0
        seT = np.zeros((896, TL_PAD), BF16NP)
        seT[0:DS, :tl] = states[t0:t0 + tl].T.astype(BF16NP)
        seT[512:512 + DE, :tl] = embeds[t0:t0 + tl].T.astype(BF16NP)

        selp, ls, le = padded[cix]
        sels.append(selp)

        mh = np.zeros((5, SMAX), BF16NP)
        real = selp >= 0
        mh[:, real] = mh_full[:, selp[real]].astype(BF16NP)

        masks = np.zeros((128, total_chunks * 128), BF16NP)
        for k in range(NT):
            slo, shi, elo, ehi, rlo, rhi, moff = tiles[k]
            a = k * 128
            lsk = ls[a:a + 128]
            lek = le[a:a + 128]
            ci = moff
            for b in range(slo, shi + 1):
                tk = 128 * b + rows
                masks[:, ci * 128:(ci + 1) * 128] = \
                    (lsk[None, :] == tk[:, None]).astype(BF16NP)
                ci += 1
            for b in range(elo, ehi + 1):
                tk = 128 * b + rows
                masks[:, ci * 128:(ci + 1) * 128] = \
                    (lek[None, :] == tk[:, None]).astype(BF16NP)
                ci += 1
            for b in range(rlo, rhi + 1):
                tk = 128 * b + rows
                masks[:, ci * 128:(ci + 1) * 128] = \
                    ((lsk[None, :] <= tk[:, None]) &
                     (tk[:, None] <= lek[None, :])).astype(BF16NP)
                ci += 1

        in_maps.append({
            "seT": seT,
            "mh": mh,
            "masks": masks,
            **shared,
        })
    return in_maps, sels, (NT, tiles, total_chunks)


def kernel(**inputs) -> np.ndarray:
    in_maps, sels, meta = prepare_in_maps(inputs)
    NT, tiles, total_chunks = meta

    key = (NT, tiles, total_chunks)
    if key not in _PROGRAM_CACHE:
        _PROGRAM_CACHE[key] = _build_program(NT, tiles, total_chunks)
        _PROGRAM_CACHE["nc"] = _PROGRAM_CACHE[key]
    nc = _PROGRAM_CACHE[key]

    from concourse.bass_utils import run_bass_kernel_spmd
    res = run_bass_kernel_spmd(nc, in_maps, core_ids=list(range(NCORES)))
    _PROGRAM_CACHE["last_res"] = res  # exec_time_ns etc, for the test harness

    out = np.zeros(S, np.float32)
    for cix in range(NCORES):
        slot_orig = sels[cix]
        m = slot_orig >= 0
        vals = np.asarray(res.results[cix]["scores"]).reshape(-1)
        out[slot_orig[m]] = vals[m]
    return out
